# revision 1
# baseline (speedup 1.0000x reference)
"""Trainium2 Bass kernel for a dense self-attention block (B=4, N=S=1024,
C=768, H=12) with an additive attention-weight bias:

    q = heads(x @ Wq.T); k = heads(x @ Wk.T); v = heads(x @ Wv.T)
    attn = softmax(attn_weight + log_softmax(scale * q k^T))
    out  = (attn @ v) @ Wo.T + bo

Key simplification: log_softmax(a) = a - lse(a) where lse is constant along
the softmax axis, so softmax(w + log_softmax(a)) == softmax(w + a) exactly.
Logits are bounded (|w + a| < ~10) so exp() is computed without max
subtraction.

Sharding: 8 cores = 4 batches x 2 head-groups (6 heads each).  Each core
computes its head-group's partial output projection; the host adds the two
halves plus the bias.

Device layout per core (all transposes of weights/activations done on host):
  xT  [768,1024]   query[b].T  (fp16)          -> SBUF [128,6,1024]
  wqT [768, 384]   (scale*Wq[g]).T (fp16)      -> SBUF [128,6,384]
  wkT, wvT         likewise (no scale)
  woT [384, 768]   Wo[:, g].T (fp16)           -> SBUF [128,3,768]
  wt  [6,1024,1024] attn_weight[b,g].T per head (fp8e4m3) -> tiles

Pipeline (head pairs, software-pipelined over s-chunks):
  warmup matmul stream (holds the PE clock-gate at 2.4 GHz through the
  input DMA fill) -> QKV (fp16, PSUM accum) -> per pair (even/odd head):
  qk matmuls (K=64, opens the PSUM group so it can never stall on the wt
  stream) + ident@wt matmul (adds the attention bias in-PE, closes the
  group) -> exp (ACT, fp16 tiles); PV matmuls of the *previous* pair ride
  inside the current pair's loop (keeps ACT fed); PV packs [v | ones] so
  the softmax denominator r falls out of the same accumulation.  PV rows
  are evacuated to SBUF immediately (PSUM recycles fast), mid-kernel 1/r
  goes through a DMA repartition chain on the sync queue, and the final
  pair uses a DMA-free path: 1/r = exp(-ln(r)) on ACT + a K=1 ones-matmul
  broadcast.  Output projection (fp16 out, host sums the two core-halves
  in fp32) contracts all 6 heads at K=128.
"""

import os
import numpy as np

B, N, C, H = 4, 1024, 768, 12
HG = 2                # head-groups (tensor-parallel factor); cores = B*HG = 8
HPG = H // HG         # heads per group = 6
D = C // H            # 64
GJ = HPG * D          # 384
P = 128
SC_ = N // P          # 8 s-chunks of 128
MQ_ = GJ // P         # 3
NCORES = B * HG
SCALE = D ** -0.5

# ---- tuning flags -----------------------------------------------------------
MM_FP32R = True            # use float32r matmul mode (4x faster, slight prec loss)
QK_FP16 = True             # fp16 for the QKV + S^T path too (fastest, less exact)
PV_FP16 = True             # fp16 for the PV + output-projection path
W_FP8 = True               # attn_weight + ident in fp8e4m3 (halves the wt
                           # DMA stream, which paces the whole kernel)
W_NP_DT = np.float16       # dtype for attn_weight transfer when not fp8
E_BUFS = 24                # exp-tile pool depth
W_BUFS = 8                 # attn-weight tile pool depth


def _mm_dt(mybir):
    if QK_FP16:
        return mybir.dt.float16
    return mybir.dt.float32r if MM_FP32R else mybir.dt.float32


def _w_mybir_dt(mybir):
    if W_FP8:
        return mybir.dt.float8e4
    return {np.float16: mybir.dt.float16,
            np.float32: mybir.dt.float32}[W_NP_DT]


def _w_np_cast(a):
    if W_FP8:
        import ml_dtypes
        return a.astype(ml_dtypes.float8_e4m3fn)
    return a.astype(W_NP_DT)


def build_program(debug_dump=False):
    """Build and compile the per-core Bass program. Returns the Bacc object."""
    import concourse.bass as bass
    import concourse.mybir as mybir
    import concourse.tile as tile
    from concourse import bacc

    nc = bacc.Bacc(
        "TRN2",
        target_bir_lowering=False,
        debug=False,
        num_devices=NCORES,
    )
    f32 = mybir.dt.float32
    wdt = _w_mybir_dt(mybir)
    # matmul-operand dtype: float32r ("rounded") or float32. All tiles that
    # feed TensorE must be produced in this dtype (BIR verifier requirement).
    cdt = _mm_dt(mybir)
    # PV-side dtype (v_aug, exp tiles, oT, woT, broadcast ones / 1/r)
    vdt = mybir.dt.float16 if PV_FP16 else cdt
    EXP = mybir.ActivationFunctionType.Exp

    xT_d = nc.dram_tensor("xT", [C, N], cdt, kind="ExternalInput").ap()
    wqT_d = nc.dram_tensor("wqT", [C, GJ], cdt, kind="ExternalInput").ap()
    wkT_d = nc.dram_tensor("wkT", [C, GJ], cdt, kind="ExternalInput").ap()
    wvT_d = nc.dram_tensor("wvT", [C, GJ], cdt, kind="ExternalInput").ap()
    woT_d = nc.dram_tensor("woT", [GJ, C], vdt, kind="ExternalInput").ap()
    wt_d = nc.dram_tensor("wt", [HPG, N, N], wdt, kind="ExternalInput").ap()
    # constant pads for v_aug (memset can't produce fp32r-rounded data)
    ident_d = nc.dram_tensor("ident", [P, P], wdt,
                             kind="ExternalInput").ap()
    vone_d = nc.dram_tensor("vone", [P, P], vdt, kind="ExternalInput").ap()
    vzero_d = nc.dram_tensor("vzero", [P, 32 * SC_], vdt,
                             kind="ExternalInput").ap()
    out_d = nc.dram_tensor("out", [N, C], f32, kind="ExternalOutput").ap()
    dbg = {}
    if debug_dump:
        for nm, shp, dt_ in (("d_qT", [P, MQ_ * N], cdt),
                             ("d_kT", [P, MQ_ * N], cdt),
                             ("d_vaug", [P, SC_ * HPG * P], vdt),
                             ("d_et0", [P, 512], vdt),
                             ("d_pso0", [P, N], f32), ("d_rt0", [P, N], f32),
                             ("d_rb0", [P, N], f32),
                             ("d_oT", [P, MQ_ * N], vdt)):
            dbg[nm] = nc.dram_tensor(nm, shp, dt_,
                                     kind="ExternalOutput").ap()

    KC = C // P      # 6 contraction chunks over C
    MQ = GJ // P     # 3 row chunks of qT/kT
    NB2 = N // 512   # 2 column chunks of 512
    SC = SC_         # 8 s chunks

    def mm(out, lhsT, rhs, start, stop):
        nc.tensor.matmul(out, lhsT, rhs, start=start, stop=stop)

    with tile.TileContext(nc) as tc:
        with (
            tc.tile_pool(name="const", bufs=1) as const_pool,
            tc.tile_pool(name="wtile", bufs=W_BUFS) as w_pool,
            tc.tile_pool(name="etile", bufs=E_BUFS) as e_pool,
            tc.tile_pool(name="rtile", bufs=4) as r_pool,
            tc.tile_pool(name="rbtile", bufs=2) as rb_pool,
            tc.tile_pool(name="vcptile", bufs=4) as vcp_pool,
            tc.tile_pool(name="outtile", bufs=2) as out_pool,
            tc.tile_pool(name="ps_s", bufs=2, space="PSUM") as psum_s,
            tc.tile_pool(name="ps_o", bufs=4, space="PSUM") as psum_o,
            tc.tile_pool(name="dram", bufs=4, space="DRAM") as dram_pool,
        ):
            # ---- load constants -------------------------------------------
            # Constants are loaded per-128-row chunk, round-robined over
            # three HWDGE queues (sync/scalar/vector); the gpsimd (SWDGE)
            # queue is reserved for the big attn_weight stream so the two
            # never compete.  x/Wq/Wk chunks (which gate the QKV matmuls)
            # are issued before Wv (only needed by the later v-projection).
            queues = [nc.sync, nc.scalar]
            ident_sb = const_pool.tile([P, P], wdt)
            nc.scalar.dma_start(ident_sb, ident_d)
            ones_sb = const_pool.tile([1, P], vdt)
            nc.sync.dma_start(ones_sb, vone_d[0:1, :])
            xT_r = xT_d.rearrange("(o p) n -> p o n", p=P)
            wq_r = wqT_d.rearrange("(o p) j -> p o j", p=P)
            wk_r = wkT_d.rearrange("(o p) j -> p o j", p=P)
            wv_r = wvT_d.rearrange("(o p) j -> p o j", p=P)
            # Per-128-row chunk loads: each chunk is DMA-contiguous per
            # partition (a whole-tensor load of the rearranged view would
            # gather 6 scattered segments per partition — measured ~5x
            # slower).  x/Wq/Wk first (they gate QKV), Wv afterwards.
            # 3 chunks per transfer: the per-DMA fixed cost (~2us incl
            # completion) dominated the fill when issued as 18 transfers.
            xg = [const_pool.tile([P, 3, N], cdt, name=f"xg{g}")
                  for g in range(2)]
            wqg = [const_pool.tile([P, 3, GJ], cdt, name=f"wqg{g}")
                   for g in range(2)]
            wkg = [const_pool.tile([P, 3, GJ], cdt, name=f"wkg{g}")
                   for g in range(2)]
            for g in range(2):
                nc.sync.dma_start(xg[g], xT_r[:, 3 * g:3 * g + 3])
                nc.scalar.dma_start(wqg[g], wq_r[:, 3 * g:3 * g + 3])
                nc.scalar.dma_start(wkg[g], wk_r[:, 3 * g:3 * g + 3])
            xT_sbs = [xg[k // 3][:, k % 3, :] for k in range(KC)]
            wq_sbs = [wqg[k // 3][:, k % 3, :] for k in range(KC)]
            wk_sbs = [wkg[k // 3][:, k % 3, :] for k in range(KC)]
            if not os.environ.get("K_SKIP_W2"):
                wvg = [const_pool.tile([P, 3, GJ], cdt, name=f"wvg{g}")
                       for g in range(2)]
                for g in range(2):
                    nc.sync.dma_start(wvg[g], wv_r[:, 3 * g:3 * g + 3])
                wv_sbs = [wvg[k // 3][:, k % 3, :] for k in range(KC)]
            woT_sb = const_pool.tile([P, MQ, C], vdt)

            # ---- PE warmup ------------------------------------------------
            # The PE clock-gate (HAM) defaults to half rate and only
            # un-throttles after ~3.4us of sustained matmul activity.  A
            # dummy matmul stream that depends only on the tiny ident tile
            # keeps the PE busy while the QKV operands stream in, so the
            # real QKV matmuls run at 2.4 GHz instead of 1.2 GHz.
            # one long accumulation group: accumulating matmuls stream
            # back-to-back (no write-after-write drain between them), which
            # is what the HAM activity monitor needs to see to un-throttle.
            warm_ps = psum_s.tile([P, N], f32, tag="ps_s")
            NWARM = 110
            for i in range(NWARM):
                mm(warm_ps[:, 0:P], ident_sb, ident_sb,
                   start=(i == 0), stop=(i == NWARM - 1))

            qT_sbs = [const_pool.tile([P, N], cdt, name=f"qT{j}")
                      for j in range(MQ)]
            kT_sbs = [const_pool.tile([P, N], cdt, name=f"kT{j}")
                      for j in range(MQ)]
            oT_sbs = [const_pool.tile([P, N], vdt, name=f"oT{j}")
                      for j in range(MQ)]
            # [v_h | 1 | 0...] (even heads use cols 0:65) /
            # [0... | 1 | v_h] (odd heads use cols 0:128, one at col 63)
            v_aug = const_pool.tile([P, SC, HPG, P], vdt)

            # even heads: [v(0:64) | one(64)]             -> r at psum row 64
            # odd heads:  [0(0:32) | one(32) | 0 | v(64:128)] -> r at row 32
            # Pads are written by COMPUTE (memzero + add on ACT, idle at
            # startup), not DMA: as DMAs these were ~15k sub-512B strided
            # segments that ran read-modify-write and monopolized all 16
            # SDMA engines for ~35us, starving the startup fill.
            for h in range(HPG) if not os.environ.get("K_SKIP_V") else []:
                if h % 2 == 0:
                    # zero cols 64:66 (col 65 is never read), then 0 -> 1
                    nc.scalar.memzero(v_aug[:, :, h, 64:66])
                    nc.scalar.add(v_aug[:, :, h, 64:65],
                                  v_aug[:, :, h, 64:65], 1.0)
                else:
                    nc.scalar.memzero(v_aug[:, :, h, 0:64])
                    nc.scalar.add(v_aug[:, :, h, 32:33],
                                  v_aug[:, :, h, 32:33], 1.0)

            # ---- QKV projections ------------------------------------------
            # Only the m=0 chunk (heads 0/1) runs up front: m=1/m=2 and the
            # v-projection are interleaved into pair 0's S^T window (which
            # is ACT-bound, leaving ~1us/step of PE slack) via fill_cb.
            def qkv_m(m):
                for wsbs, dsts, nm in ((wq_sbs, qT_sbs, "q"),
                                       (wk_sbs, kT_sbs, "k")):
                    ps = psum_s.tile([P, N], f32, tag="ps_s",
                                     name=f"qkv_{nm}{m}")
                    for nb in range(NB2):
                        ncol = slice(nb * 512, (nb + 1) * 512)
                        for kc in range(KC):
                            mm(ps[:, ncol],
                               wsbs[kc][:, m * P:(m + 1) * P],
                               xT_sbs[kc][:, ncol],
                               start=(kc == 0), stop=(kc == KC - 1))
                    # NB: DVE CAST (f32 psum -> f16) mis-strides on HW;
                    # ScalarE casts fine.
                    if cdt != mybir.dt.float16:
                        nc.vector.tensor_copy(dsts[m][:], ps)
                    else:
                        nc.scalar.copy(dsts[m][:], ps)

            qkv_m(0)

            def emit_v(s0=0, s1=SC):
                # v-projection: runs on psum_o tiles (idle before the first
                # PV) so it never competes with the S^T loop's ps_s slots;
                # emitted inside pair 0's window to stay off the critical
                # path between QKV and the exp stream.
                for sc in range(s0, s1):
                    ps = psum_o.tile([P, 512], f32, tag="ps_o",
                                     name=f"ps_v{sc}")
                    for kc in range(KC):
                        mm(ps[:, :GJ],
                           xT_sbs[kc][:, sc * P:(sc + 1) * P],
                           wv_sbs[kc][:, :],
                           start=(kc == 0), stop=(kc == KC - 1))
                    vsrc = ps[:, :GJ].rearrange("p (h d) -> p h d", d=D)
                    nc.scalar.copy(v_aug[:, sc, 0:HPG:2, 0:64],
                                   vsrc[:, 0:HPG:2, :])
                    nc.scalar.copy(v_aug[:, sc, 1:HPG:2, 64:128],
                                   vsrc[:, 1:HPG:2, :])

            if os.environ.get("K_SKIP_ATTN") and not os.environ.get("K_SKIP_V"):
                emit_v()

            if debug_dump:
                if os.environ.get("K_QT_F32"):
                    tq = r_pool.tile([P, N], f32, tag="dbgcp")
                    nc.scalar.copy(tq, qT_sbs[0])
                    nc.sync.dma_start(dbg["d_pso0"], tq)
                for j in range(MQ):
                    nc.sync.dma_start(dbg["d_qT"][:, j * N:(j + 1) * N],
                                      qT_sbs[j])
                    nc.sync.dma_start(dbg["d_kT"][:, j * N:(j + 1) * N],
                                      kT_sbs[j])
                if not os.environ.get("K_SKIP_V"):
                    nc.sync.dma_start(dbg["d_vaug"],
                                      v_aug.rearrange("p a b c -> p (a b c)"))

            # gpsimd-queue barrier: a tiny DMA that depends on the x load.
            # The SDMA engines round-robin across queues, so letting the wt
            # stream start at t=0 steals bandwidth from the startup-critical
            # x/Wq/Wk fill; this holds all gpsimd-queue DMAs (woT + wt)
            # until x has landed.
            # Gate the wt stream behind the startup fill with a REAL
            # dependency: pre-occupy every w_pool slot with a dummy tile
            # whose (tiny, casting) write waits on the last x chunk.  The
            # real wt DMAs rotate into these slots, so their transfers are
            # semaphore-ordered after the fill — the list scheduler cannot
            # hoist them (which it does to both a dependency-gate DMA and
            # tile_wait_until holds, letting the wt stream steal ~2/3 of
            # the fill bandwidth).
            for i in range(W_BUFS):
                wgate = w_pool.tile([P, N], wdt, tag="wt",
                                    name=f"wgate{i}")
                nc.gpsimd.dma_start(wgate[0:1, 0:64],
                                    xT_sbs[KC - 1][0:1, 0:64])
            if not os.environ.get("K_SKIP_W2"):
                nc.gpsimd.dma_start(woT_sb,
                                    woT_d.rearrange("(o p) c -> p o c", p=P))

            # ---- attention, pairwise-pipelined heads ----------------------
            # Heads are processed in even/odd pairs per s-chunk.  Matmuls
            # are grouped by array row-config (all K=128 ident/PV first,
            # then the K=64 qk block) because switching the contraction
            # config between consecutive matmuls costs a ~105ns pipeline
            # bubble.  The even head's qk (rows 0-63) and the odd head's
            # (rows 64-127) are issued back-to-back: they occupy disjoint
            # row groups of the PE array and execute concurrently.
            # PV matmuls of the *previous* pair are interleaved into the
            # current pair's S^T loop so the ACT exp stream never starves
            # behind a dense PV block.
            # Per-step PV emission plan: pair p-1's PV matmuls ride inside
            # pair p's S^T loop, compressed so the PV accumulation closes by
            # step 5 — its norm chain then completes during steps 6-7
            # instead of piling up after the pair ends (which previously
            # pushed three pairs' norm chains into the kernel tail).
            PV_PLAN = [0, 1, 2, 2, 2, 1, 0, 0]

            def st_pair(hp, pv_cb=None, fill_cb=None):
                j = hp // 2
                qe, ke = qT_sbs[j][0:64, :], kT_sbs[j][0:64, :]
                qo, ko = qT_sbs[j][64:128, :], kT_sbs[j][64:128, :]
                ets_e, ets_o = [], []
                pv_next = 0
                for sc in range(SC):
                    scol = slice(sc * P, (sc + 1) * P)
                    # For the first pair, hold the wt DMAs back in the
                    # scheduler's model so they are ORDERED after the
                    # startup x/Wq/Wk fill: the list scheduler otherwise
                    # places them first (they have no input deps) and the
                    # SDMA round-robin then starves the fill.
                    import contextlib
                    hold = tc.tile_wait_until(0.014) if hp == 0 \
                        else contextlib.nullcontext()
                    with hold:
                        wt_e = w_pool.tile([P, N], wdt, tag="wt")
                        nc.gpsimd.dma_start(wt_e, wt_d[hp, scol, :])
                        wt_o = w_pool.tile([P, N], wdt, tag="wt")
                        nc.scalar.dma_start(wt_o, wt_d[hp + 1, scol, :])
                    ps_e = psum_s.tile([P, N], f32, tag="ps_s")
                    ps_o = psum_s.tile([P, N], f32, tag="ps_s")
                    # qk first (start=True): has no DMA dependency, so the
                    # group never head-of-line-blocks on the wt stream; the
                    # ident matmul adds wt and closes the group.
                    for nb in range(NB2):
                        ncol = slice(nb * 512, (nb + 1) * 512)
                        mm(ps_e[:, ncol], ke[:, scol], qe[:, ncol],
                           start=True, stop=False)
                        mm(ps_o[:, ncol], ko[:, scol], qo[:, ncol],
                           start=True, stop=False)
                    # PV matmuls of the previous pair, per the plan above.
                    if pv_cb is not None:
                        for _ in range(PV_PLAN[sc]):
                            pv_cb(pv_next)
                            pv_next += 1
                    for ps, wt_t in ((ps_e, wt_e), (ps_o, wt_o)):
                        for nb in range(NB2):
                            ncol = slice(nb * 512, (nb + 1) * 512)
                            mm(ps[:, ncol], ident_sb, wt_t[:, ncol],
                               start=False, stop=True)
                    et_e = e_pool.tile([P, N], vdt, tag="et")
                    nc.scalar.activation(et_e, ps_e, EXP)
                    et_o = e_pool.tile([P, N], vdt, tag="et")
                    nc.scalar.activation(et_o, ps_o, EXP)
                    if debug_dump and hp == 0 and sc == 0:
                        nc.sync.dma_start(dbg["d_et0"], et_e[:, 0:512])
                    ets_e.append(et_e)
                    ets_o.append(et_o)
                    if fill_cb is not None:
                        fill_cb(sc)
                if pv_cb is not None:
                    while pv_next < SC:
                        pv_cb(pv_next)
                        pv_next += 1
                return ets_e, ets_o

            def make_pv(hp, ets_pair):
                """Allocate PSUM halves for pair hp and return a per-sc
                emitter plus the halves for the two norm chains."""
                halves = {}
                for h, _ in ets_pair:
                    for nb in range(NB2):
                        halves[(h, nb)] = psum_o.tile([P, 512], f32,
                                                      tag="ps_o",
                                                      name=f"pso_h{h}_n{nb}")

                def emit(sc):
                    for h, etiles in ets_pair:
                        even = (h % 2 == 0)
                        lh = (v_aug[:, sc, h, 0:65] if even
                              else v_aug[:, sc, h, 0:P])
                        for nb in range(NB2):
                            ncol = slice(nb * 512, (nb + 1) * 512)
                            pso = halves[(h, nb)]
                            po = (pso[0:65, :] if even else pso[:, :])
                            mm(po, lh, etiles[sc][:, ncol],
                               start=(sc == 0), stop=(sc == SC - 1))

                h_e, h_o = ets_pair[0][0], ets_pair[1][0]
                return emit, ([halves[(h_e, 0)], halves[(h_e, 1)]],
                              [halves[(h_o, 0)], halves[(h_o, 1)]])

            def norm_chain(h, halves, last=False):
                off = (h % 2) * 64
                even = (h % 2 == 0)
                rrow = 64 if even else 32
                # One merged DMA round-trip per head (both 512-col halves in
                # a single [1,1024] chain): 4 hops instead of 8.  The chain
                # hops have upstream DVE dependencies, so they would
                # head-of-line-block any queue with a live stream behind
                # them: mid-kernel that means keeping them OFF gpsimd (wt
                # stream) and scalar (exp stream) — both heads ride sync.
                # At the tail the wt stream is done, so the odd head's chain
                # moves to gpsimd and runs concurrently.
                dq = (nc.sync if even else nc.scalar) if last else nc.sync
                rb = rb_pool.tile([P, N], f32, tag="rb")
                r_t = r_pool.tile([P, N], f32, tag="r")
                # Evacuate the PV rows + r row to SBUF immediately: the PSUM
                # tiles recycle in <1us instead of being held through the
                # whole 1/r chain (which previously stalled the next pair's
                # PV allocation and let the PE clock-gate re-throttle).
                # r rows first: they gate the 1/r DMA chain; the bulk copies
                # then overlap the chain's first hops.
                vcp = vcp_pool.tile([P, N], f32, tag="vcp")
                for nb, pso in enumerate(halves):
                    ncol = slice(nb * 512, (nb + 1) * 512)
                    nc.vector.tensor_copy(r_t[rrow:rrow + 1, ncol],
                                          pso[rrow:rrow + 1, :])
                for nb, pso in enumerate(halves):
                    ncol = slice(nb * 512, (nb + 1) * 512)
                    nc.vector.tensor_copy(vcp[off:off + 64, ncol],
                                          pso[off:off + 64, :])
                # repartition [1,N] -> [128,N/128] for a fast reciprocal,
                # then broadcast 1/r back across the 64 output partitions
                # (cross-partition reshapes must round-trip through DRAM)
                rd1 = dram_pool.tile([1, N], f32, tag="rd1")
                dq.dma_start(rd1, r_t[rrow:rrow + 1, :])
                rsq = r_pool.tile([P, N // P], f32, tag="rsq")
                dq.dma_start(
                    rsq, rd1.rearrange("one (p o) -> (one p) o", p=P))
                nc.vector.reciprocal(rsq, rsq)
                rd2 = dram_pool.tile([1, N], f32, tag="rd2")
                dq.dma_start(
                    rd2.rearrange("one (p o) -> (one p) o", p=P), rsq)
                dq.dma_start(rb[off:off + 64, :],
                             rd2[0:1, :].partition_broadcast(64))
                for nb in range(NB2):
                    ncol = slice(nb * 512, (nb + 1) * 512)
                    nc.vector.tensor_mul(
                        oT_sbs[h // 2][off:off + 64, ncol],
                        vcp[off:off + 64, ncol],
                        rb[off:off + 64, ncol])
                if debug_dump and h == 0:
                    tmpd = r_pool.tile([P, N], f32, tag="dbgcp")
                    for nb, pso in enumerate(halves):
                        nc.scalar.copy(tmpd[0:P, nb * 512:(nb + 1) * 512],
                                       pso[0:P, :])
                    nc.sync.dma_start(dbg["d_pso0"], tmpd)
                    nc.sync.dma_start(dbg["d_rb0"], rb)

            def norm_tail(hp, halves_e, halves_o):
                """DMA-free norm for the final pair: 1/r = exp(-ln(r)) on
                ACT directly on the [1,N] r-row (Log and Exp share one
                table set), then a K=1 ones-matmul broadcasts 1/r across
                the 64 output partitions via PSUM.  Avoids the ~7us DRAM
                round-trip that otherwise sits on the kernel tail."""
                LOG = mybir.ActivationFunctionType.Ln
                # ALL r-row copies first: both LN inputs then become ready
                # before the first LN finishes, so the scheduler emits
                # [LN, LN, EXP, EXP] (2 table-set loads) instead of
                # interleaving per head (4 loads); the bulk vcp copies run
                # during the activations.
                infos = []
                for h, halves in ((hp, halves_e), (hp + 1, halves_o)):
                    off = (h % 2) * 64
                    rrow = 64 if h % 2 == 0 else 32
                    vcp = vcp_pool.tile([P, N], f32, tag="vcp",
                                        name=f"vcpt{h}")
                    r_t = r_pool.tile([P, N], f32, tag="r", name=f"rt{h}")
                    for nb, pso in enumerate(halves):
                        ncol = slice(nb * 512, (nb + 1) * 512)
                        nc.vector.tensor_copy(r_t[rrow:rrow + 1, ncol],
                                              pso[rrow:rrow + 1, :])
                    infos.append((h, off, rrow, vcp, r_t))
                for (h, off, rrow, vcp, r_t), halves in zip(
                        infos, (halves_e, halves_o)):
                    for nb, pso in enumerate(halves):
                        ncol = slice(nb * 512, (nb + 1) * 512)
                        nc.vector.tensor_copy(vcp[off:off + 64, ncol],
                                              pso[off:off + 64, :])
                lns = []
                for h, off, rrow, vcp, r_t in infos:
                    rln = r_pool.tile([1, N], f32, tag="rsq",
                                      name=f"rln{h}")
                    nc.scalar.activation(rln, r_t[rrow:rrow + 1, :], LOG)
                    lns.append(rln)
                rinvs = []
                for (h, off, rrow, vcp, r_t), rln in zip(infos, lns):
                    rinv = r_pool.tile([1, N], vdt, tag="rfl",
                                       name=f"rinv{h}")
                    nc.scalar.activation(rinv, rln, EXP, scale=-1.0)
                    rinvs.append(rinv)
                for (h, off, rrow, vcp, r_t), rinv in zip(infos, rinvs):
                    for nb in range(NB2):
                        ncol = slice(nb * 512, (nb + 1) * 512)
                        rbp = psum_o.tile([P, 512], f32, tag="ps_o",
                                          name=f"rbp{h}_{nb}")
                        mm(rbp[off:off + 64, :], ones_sb[0:1, 0:64],
                           rinv[0:1, ncol], start=True, stop=True)
                        nc.vector.tensor_mul(
                            oT_sbs[h // 2][off:off + 64, ncol],
                            vcp[off:off + 64, ncol],
                            rbp[off:off + 64, :])

            # software pipeline over pairs: pair p's S^T loop carries the
            # PV matmuls of pair p-1 (via pv_cb); the norm chains for p-1
            # are emitted right after, overlapping pair p's stream.  Only
            # the last pair's PV runs as a dense block at the end.
            prev = None
            for hp in range(0, HPG, 2) if not os.environ.get("K_SKIP_ATTN") else []:
                if prev is not None:
                    p_hp, p_ets = prev
                    pv_emit, (h0_e, h0_o) = make_pv(p_hp, p_ets)
                    ets_e, ets_o = st_pair(hp, pv_cb=pv_emit)
                    norm_chain(p_hp, h0_e)
                    norm_chain(p_hp + 1, h0_o)
                else:
                    fills = {1: lambda: qkv_m(1), 3: lambda: qkv_m(2)}
                    if not os.environ.get("K_SKIP_V"):
                        fills[5] = lambda: emit_v(0, 4)
                        fills[7] = lambda: emit_v(4, 8)
                    ets_e, ets_o = st_pair(
                        hp, fill_cb=lambda sc: fills.get(sc, lambda: None)())
                prev = (hp, [(hp, ets_e), (hp + 1, ets_o)])
            def oproj_mms(nb, ps0, ps1, j3s, start, stop):
                for cb, ps in ((0, ps0), (1, ps1)):
                    cw = 512 if cb == 0 else C - 512
                    for j3 in j3s:
                        mm(ps[:, 0:cw],
                           oT_sbs[j3][:, nb * P:(nb + 1) * P],
                           woT_sb[:, j3, cb * 512:cb * 512 + cw],
                           start=(start and j3 == j3s[0]),
                           stop=(stop and j3 == j3s[-1]))

            def oproj_evac(nb, ps0, ps1):
                ob = out_pool.tile([P, C], f32, tag="ob")
                nc.vector.tensor_copy(ob[:, 0:512], ps0)
                nc.vector.tensor_copy(ob[:, 512:C], ps1[:, 0:C - 512])
                nc.sync.dma_start(
                    out_d.rearrange("(o p) c -> o p c", p=P)[nb], ob)

            if prev is not None:
                p_hp, p_ets = prev
                pv_emit, (h0_e, h0_o) = make_pv(p_hp, p_ets)
                for sc in range(SC):
                    pv_emit(sc)
                # Emit the first two output-projection chunks' j3=0/1
                # partials (ps_s banks, free after the last exps) BEFORE
                # the final norm: they sit ahead of the rbp broadcast
                # matmuls in the PE FIFO and keep the PE busy/warm while
                # the 1/r chain resolves (otherwise the broadcasts
                # head-of-line-block the queue for ~5us and the clock-gate
                # re-throttles the whole projection).
                pre = []
                for nb in range(2):
                    psw = psum_s.tile([P, N], f32, tag="ps_s",
                                      name=f"pow_{nb}")
                    ps0, ps1 = psw[:, 0:512], psw[:, 512:1024]
                    oproj_mms(nb, ps0, ps1, [0, 1], True, False)
                    pre.append((ps0, ps1))
                norm_tail(p_hp, h0_e, h0_o)
            else:
                pre = []

            if debug_dump and not os.environ.get("K_SKIP_ATTN"):
                for j in range(MQ):
                    nc.sync.dma_start(dbg["d_oT"][:, j * N:(j + 1) * N],
                                      oT_sbs[j])

            # ---- output projection ----------------------------------------
            for nb in range(SC) if not os.environ.get("K_SKIP_ATTN") else []:
                if nb < len(pre):
                    ps0, ps1 = pre[nb]
                    oproj_mms(nb, ps0, ps1, [2], False, True)
                else:
                    ps0 = psum_o.tile([P, 512], f32, tag="ps_o",
                                      name=f"po0_{nb}")
                    ps1 = psum_o.tile([P, 512], f32, tag="ps_o",
                                      name=f"po1_{nb}")
                    oproj_mms(nb, ps0, ps1, [0, 1, 2], True, True)
                oproj_evac(nb, ps0, ps1)

    nc.compile()
    return nc


_PROG = None


def _get_prog():
    global _PROG
    if _PROG is None:
        _PROG = build_program()
    return _PROG


def make_in_maps(query, attn_weight, Wq, Wk, Wv, Wo):
    query = np.asarray(query, dtype=np.float32)
    attn_weight = np.asarray(attn_weight, dtype=np.float32)
    Wq = np.asarray(Wq, dtype=np.float32)
    Wk = np.asarray(Wk, dtype=np.float32)
    Wv = np.asarray(Wv, dtype=np.float32)
    Wo = np.asarray(Wo, dtype=np.float32)

    vnp = np.float16 if PV_FP16 else np.float32
    cnp = np.float16 if QK_FP16 else np.float32
    in_maps = []
    for b in range(B):
        xT = np.ascontiguousarray(query[b].T).astype(cnp)
        for g in range(HG):
            rows = slice(g * GJ, (g + 1) * GJ)
            wqT = np.ascontiguousarray((SCALE * Wq[rows, :]).T).astype(cnp)
            wkT = np.ascontiguousarray(Wk[rows, :].T).astype(cnp)
            wvT = np.ascontiguousarray(Wv[rows, :].T).astype(cnp)
            woT = np.ascontiguousarray(Wo[:, rows].T).astype(vnp)
            wt = _w_np_cast(np.ascontiguousarray(
                attn_weight[b, g * HPG:(g + 1) * HPG].transpose(0, 2, 1)))
            in_maps.append({
                "xT": xT, "wqT": wqT, "wkT": wkT, "wvT": wvT,
                "woT": woT, "wt": wt,
                "ident": _w_np_cast(np.eye(P, dtype=np.float32)),
                "vone": np.ones((P, P), vnp),
                "vzero": np.zeros((P, 32 * SC_), vnp),
            })
    return in_maps


def run(inputs, trace=False, **spmd_kwargs):
    """Execute on 8 cores; returns (full_output, BassKernelResults)."""
    from concourse import bass_utils

    nc = _get_prog()
    in_maps = make_in_maps(inputs["query"], inputs["attn_weight"],
                           inputs["Wq"], inputs["Wk"], inputs["Wv"],
                           inputs["Wo"])
    res = bass_utils.run_bass_kernel_spmd(
        nc, in_maps, core_ids=list(range(NCORES)), trace=trace, **spmd_kwargs)
    bo = np.asarray(inputs["bo"], dtype=np.float32)
    full = np.empty((B, N, C), dtype=np.float32)
    for b in range(B):
        full[b] = (res.results[2 * b]["out"].astype(np.float32)
                   + res.results[2 * b + 1]["out"].astype(np.float32) + bo)
    return full, res


def kernel(**inputs):
    full, _ = run(inputs, trace=False)
    return full



# revision 13
# speedup vs baseline: 1.0305x; 1.0305x over previous
"""Trainium2 Bass kernel for a dense self-attention block (B=4, N=S=1024,
C=768, H=12) with an additive attention-weight bias:

    q = heads(x @ Wq.T); k = heads(x @ Wk.T); v = heads(x @ Wv.T)
    attn = softmax(attn_weight + log_softmax(scale * q k^T))
    out  = (attn @ v) @ Wo.T + bo

Math simplifications (exact):
  softmax(w + log_softmax(a)) == softmax(w + a)          (lse shift invariance)
  exp(w + a) == exp(a) * exp(w)  with exp(w) precomputed on HOST.

The second identity removes the in-PE bias-add (an identity-matmul per
attention tile that cost ~25% of all TensorE columns in v1): the device now
computes et = exp(qk) on ACT and multiplies elementwise by the streamed
exp(w) tiles on the otherwise-idle Vector engine.

Sharding: 8 cores = 4 batches x 2 head-groups (6 heads each).  Each core
computes its head-group's partial output projection in fp16; the host adds
the two halves plus the bias in fp32.

Per-core schedule (head pairs; S^T loops are ACT/PE co-paced):
  fill (x / wqk chunks, QKV-m0 matmuls stream behind the per-chunk DMAs,
  short PE warmup holds the HAM clock-gate) ->
  pair 0: qk+exp+mul stream, v-projection + qkv m1 fills ->
  pair 1: + PV(pair0) burst + qkv m2 fills + pair0 norm chains ->
  pair 2: + PV(pair1) burst + pair1 norms + PV(pair2) self-lag ->
  tail: PV2 leftovers, dual DMA-repartition norm chains (sync+gpsimd),
  output projection streamed per-nb (j3=0,1 pre-run while 1/r resolves),
  fp16 output DMA.
"""

import os
import numpy as np

B, N, C, H = 4, 1024, 768, 12
HG = 2                # head-groups (tensor-parallel factor); cores = B*HG = 8
HPG = H // HG         # heads per group = 6
D = C // H            # 64
GJ = HPG * D          # 384
P = 128
SC = N // P           # 8 s-chunks of 128
MQ = GJ // P          # 3 row chunks of qT/kT
KC = C // P           # 6 contraction chunks over C
NB2 = N // 512        # 2 column chunks of 512
NCORES = B * HG
SCALE = D ** -0.5

NWARM = int(os.environ.get("K_NWARM", "24"))
EW_BUFS = 5
ET_BUFS = 20
ERAW_BUFS = 6


def build_program():
    """Build and compile the per-core Bass program. Returns the Bacc object."""
    import concourse.bass as bass
    import concourse.mybir as mybir
    import concourse.tile as tile
    from concourse import bacc

    nc = bacc.Bacc(
        "TRN2",
        target_bir_lowering=False,
        debug=False,
        num_devices=NCORES,
    )
    f32 = mybir.dt.float32
    f16 = mybir.dt.float16
    EXP = mybir.ActivationFunctionType.Exp

    xT_d = nc.dram_tensor("xT", [C, N], f16, kind="ExternalInput").ap()
    wqk_d = nc.dram_tensor("wqk", [C, 2 * GJ], f16, kind="ExternalInput").ap()
    wvT_d = nc.dram_tensor("wvT", [C, GJ], f16, kind="ExternalInput").ap()
    woT_d = nc.dram_tensor("woT", [GJ, C], f16, kind="ExternalInput").ap()
    ew_d = nc.dram_tensor("ew", [HPG, N, N], f16, kind="ExternalInput").ap()
    out_d = nc.dram_tensor("out", [N, C], f16, kind="ExternalOutput").ap()

    def mm(out, lhsT, rhs, start, stop):
        nc.tensor.matmul(out, lhsT, rhs, start=start, stop=stop)

    with tile.TileContext(nc) as tc:
        with (
            tc.tile_pool(name="const", bufs=1) as const_pool,
            tc.tile_pool(name="ewtile", bufs=EW_BUFS) as ew_pool,
            tc.tile_pool(name="eraw", bufs=ERAW_BUFS) as eraw_pool,
            tc.tile_pool(name="etile", bufs=ET_BUFS) as e_pool,
            tc.tile_pool(name="rtile", bufs=4) as r_pool,
            tc.tile_pool(name="rbtile", bufs=2) as rb_pool,
            tc.tile_pool(name="vcptile", bufs=4) as vcp_pool,
            tc.tile_pool(name="outtile", bufs=2) as out_pool,
            tc.tile_pool(name="ps_s", bufs=2, space="PSUM") as psum_s,
            tc.tile_pool(name="ps_o", bufs=4, space="PSUM") as psum_o,
            tc.tile_pool(name="dram", bufs=4, space="DRAM") as dram_pool,
        ):
            # ---- constants / fill -----------------------------------------
            # x chunks on sync, wqk chunks on scalar: these gate QKV-m0 and
            # stream per-128-row chunk so the m0 matmuls can run behind the
            # fill.  wv/woT ride the vector queue gated behind the last x
            # chunk (per-queue FIFO + sequencer-side semaphore wait), so
            # they never steal SDMA round-robin share from the critical
            # fill.  The big exp(attn_weight) stream rides gpsimd (SWDGE),
            # gated the same way via dummy pool tiles.
            warm_sb = const_pool.tile([P, P], f16)
            nc.gpsimd.memset(warm_sb, 0.0)

            xg = [const_pool.tile([P, N], f16, name=f"xg{k}")
                  for k in range(KC)]
            wqkg = [const_pool.tile([P, 2 * GJ], f16, name=f"wqkg{k}")
                    for k in range(KC)]
            xT_r = xT_d.rearrange("(o p) n -> o p n", p=P)
            wqk_r = wqk_d.rearrange("(o p) j -> o p j", p=P)
            for k in range(KC):
                nc.sync.dma_start(xg[k], xT_r[k])
                nc.scalar.dma_start(wqkg[k], wqk_r[k])

            # wv / woT on the sync queue, gated behind the last x chunk (a
            # tiny DMA with a real data dependency — the list scheduler
            # cannot hoist the loads ahead of the fill, and same-queue
            # FIFO then orders the transfers after the gate fires).
            wvg = [const_pool.tile([P, 3, GJ], f16, name=f"wvg{g}")
                   for g in range(2)]
            woT_sb = const_pool.tile([P, MQ, C], f16)
            nc.sync.dma_start(wvg[0][0:1, 0:1, 0:64], xg[KC - 1][0:1, 0:64])
            wv_r = wvT_d.rearrange("(o p) j -> p o j", p=P)
            for g in range(2):
                nc.sync.dma_start(wvg[g], wv_r[:, 3 * g:3 * g + 3])
            nc.sync.dma_start(woT_sb,
                              woT_d.rearrange("(o p) c -> p o c", p=P))
            wv_sbs = [wvg[k // 3][:, k % 3, :] for k in range(KC)]

            # gate the ew stream: dummy pool tiles whose (tiny) write waits
            # on the last x chunk; real ew DMAs rotate into these slots and
            # are therefore semaphore-ordered after the fill.
            for i in range(EW_BUFS):
                g = ew_pool.tile([P, 2, N], f16, tag="ew", name=f"ewgate{i}")
                nc.gpsimd.dma_start(g[0:1, 0:1, 0:64],
                                    xg[KC - 1][0:1, 0:64])
            # ew view: [sc, p, h, n]
            ew_r = ew_d.rearrange("h (c p) n -> c p h n", p=P)

            # ---- PE warmup ------------------------------------------------
            # Short zero-matmul stream: covers the ~2us between the framework
            # preamble and the first x chunk landing, so the HAM clock-gate
            # sees sustained activity leading into the QKV-m0 stream.
            warm_ps = psum_s.tile([P, N], f32, tag="ps_s")
            for i in range(NWARM):
                mm(warm_ps[:, 0:P], warm_sb, warm_sb,
                   start=(i == 0), stop=(i == NWARM - 1))

            qT_sbs = [const_pool.tile([P, N], f16, name=f"qT{j}")
                      for j in range(MQ)]
            kT_sbs = [const_pool.tile([P, N], f16, name=f"kT{j}")
                      for j in range(MQ)]
            oT_sbs = [const_pool.tile([P, N], f16, name=f"oT{j}")
                      for j in range(MQ)]
            # [v_h | 1 | 0...] (even heads use cols 0:65) /
            # [0... | 1 | 0 | v_h] (odd heads use cols 0:128, one at col 32)
            v_aug = const_pool.tile([P, SC, HPG, P], f16)
            for h in range(HPG):
                if h % 2 == 0:
                    nc.scalar.memzero(v_aug[:, :, h, 64:66])
                    nc.scalar.add(v_aug[:, :, h, 64:65],
                                  v_aug[:, :, h, 64:65], 1.0)
                else:
                    nc.scalar.memzero(v_aug[:, :, h, 0:64])
                    nc.scalar.add(v_aug[:, :, h, 32:33],
                                  v_aug[:, :, h, 32:33], 1.0)

            # ---- QKV projections ------------------------------------------
            # m0 runs interleaved with the fill (per-kc chunks land in
            # order; the matmul groups stream right behind them).  m1/m2
            # are emitted inside pair 0/1's S^T windows.
            def qkv_m0():
                ps_q = psum_s.tile([P, N], f32, tag="ps_s", name="qkv_q0")
                ps_k = psum_s.tile([P, N], f32, tag="ps_s", name="qkv_k0")
                for kc in range(KC):
                    for ps, j0 in ((ps_q, 0), (ps_k, GJ)):
                        for nb in range(NB2):
                            ncol = slice(nb * 512, (nb + 1) * 512)
                            mm(ps[:, ncol],
                               wqkg[kc][:, j0:j0 + P],
                               xg[kc][:, ncol],
                               start=(kc == 0), stop=(kc == KC - 1))
                # NB: DVE CAST (f32 psum -> f16) mis-strides on HW;
                # ScalarE casts fine.
                nc.scalar.copy(kT_sbs[0][:], ps_k)
                nc.scalar.copy(qT_sbs[0][:], ps_q)

            qkv_m0()

            def qkv_m1(m, which):
                """Emit one of q/k for row-chunk m (1 psum slot, scalar cast)."""
                j0 = m * P if which == "q" else GJ + m * P
                dst = qT_sbs[m] if which == "q" else kT_sbs[m]
                ps = psum_s.tile([P, N], f32, tag="ps_s",
                                 name=f"qkv_{which}{m}")
                for nb in range(NB2):
                    ncol = slice(nb * 512, (nb + 1) * 512)
                    for kc in range(KC):
                        mm(ps[:, ncol], wqkg[kc][:, j0:j0 + P],
                           xg[kc][:, ncol],
                           start=(kc == 0), stop=(kc == KC - 1))
                nc.scalar.copy(dst[:], ps)

            def emit_v(sc):
                # v-projection for one s-chunk on a ps_o slot; ScalarE
                # scatters the result into v_aug (even cols 0:64 / odd
                # cols 64:128 per head).
                ps = psum_o.tile([P, 512], f32, tag="ps_o", name=f"ps_v{sc}")
                for kc in range(KC):
                    mm(ps[:, :GJ],
                       xg[kc][:, sc * P:(sc + 1) * P],
                       wv_sbs[kc][:, :],
                       start=(kc == 0), stop=(kc == KC - 1))
                vsrc = ps[:, :GJ].rearrange("p (h d) -> p h d", d=D)
                nc.scalar.copy(v_aug[:, sc, 0:HPG:2, 0:64],
                               vsrc[:, 0:HPG:2, :])
                nc.scalar.copy(v_aug[:, sc, 1:HPG:2, 64:128],
                               vsrc[:, 1:HPG:2, :])

            # ---- attention pair loop --------------------------------------
            def st_pair(hp, pv_sched=None, pv_emit=None, hooks=None,
                        ets_out=None):
                """One even/odd head pair's qk -> exp -> (x ew) stream.
                pv_sched[sc] PV groups of pv_emit are interleaved per step;
                hooks[sc] emits arbitrary extra work (fills, norms).
                ets_out=(list_e, list_o) lets hooks see tiles mid-emission
                (needed for the last pair's self-lag PV)."""
                j = hp // 2
                qe, ke = qT_sbs[j][0:64, :], kT_sbs[j][0:64, :]
                qo, ko = qT_sbs[j][64:128, :], kT_sbs[j][64:128, :]
                ets_e, ets_o = ([], []) if ets_out is None else ets_out
                pv_next = 0
                for sc in range(SC):
                    scol = slice(sc * P, (sc + 1) * P)
                    ew_t = ew_pool.tile([P, 2, N], f16, tag="ew")
                    nc.gpsimd.dma_start(ew_t, ew_r[sc][:, hp:hp + 2, :])
                    ps_e = psum_s.tile([P, N], f32, tag="ps_s")
                    ps_o = psum_s.tile([P, N], f32, tag="ps_s")
                    # each 512-col psum region is written by exactly one
                    # matmul -> start+stop per region
                    for nb in range(NB2):
                        ncol = slice(nb * 512, (nb + 1) * 512)
                        mm(ps_e[:, ncol], ke[:, scol], qe[:, ncol],
                           start=True, stop=True)
                    for nb in range(NB2):
                        ncol = slice(nb * 512, (nb + 1) * 512)
                        mm(ps_o[:, ncol], ko[:, scol], qo[:, ncol],
                           start=True, stop=True)
                    if pv_emit is not None and pv_sched is not None:
                        for _ in range(pv_sched[sc]):
                            pv_emit(pv_next)
                            pv_next += 1
                    er_e = eraw_pool.tile([P, N], f16, tag="eraw")
                    nc.scalar.activation(er_e, ps_e, EXP)
                    er_o = eraw_pool.tile([P, N], f16, tag="eraw")
                    nc.scalar.activation(er_o, ps_o, EXP)
                    et_e = e_pool.tile([P, N], f16, tag="et")
                    nc.vector.tensor_mul(et_e, er_e, ew_t[:, 0, :])
                    et_o = e_pool.tile([P, N], f16, tag="et")
                    nc.vector.tensor_mul(et_o, er_o, ew_t[:, 1, :])
                    ets_e.append(et_e)
                    ets_o.append(et_o)
                    if hooks is not None and sc in hooks:
                        for fn in hooks[sc]:
                            fn()
                return ets_e, ets_o, pv_next

            def make_pv(hp, ets_pair):
                halves = {}
                for h, _ in ets_pair:
                    for nb in range(NB2):
                        halves[(h, nb)] = psum_o.tile([P, 512], f32,
                                                      tag="ps_o",
                                                      name=f"pso_h{h}_n{nb}")

                def emit(sc):
                    for h, etiles in ets_pair:
                        even = (h % 2 == 0)
                        lh = (v_aug[:, sc, h, 0:65] if even
                              else v_aug[:, sc, h, 0:P])
                        for nb in range(NB2):
                            ncol = slice(nb * 512, (nb + 1) * 512)
                            pso = halves[(h, nb)]
                            po = (pso[0:65, :] if even else pso[:, :])
                            mm(po, lh, etiles[sc][:, ncol],
                               start=(sc == 0), stop=(sc == SC - 1))

                h_e, h_o = ets_pair[0][0], ets_pair[1][0]
                return emit, ([halves[(h_e, 0)], halves[(h_e, 1)]],
                              [halves[(h_o, 0)], halves[(h_o, 1)]])

            # Norm chain, split into 3 stages so that when it is emitted
            # inside a pair window, each DVE op's inputs are already
            # resolved when it reaches the head of the strict-FIFO DVE
            # queue (an unresolved reciprocal would otherwise block the
            # exp-tile multiply stream for ~2us).
            def norm_a(h, halves, dq):
                """Evacuate r row + PV rows to SBUF, launch the
                repartition DMA (queue dq).  Frees the psum halves."""
                off = (h % 2) * 64
                rrow = 64 if h % 2 == 0 else 32
                r_t = r_pool.tile([P, N], f32, tag="r")
                vcp = vcp_pool.tile([P, N], f32, tag="vcp")
                for nb, pso in enumerate(halves):
                    ncol = slice(nb * 512, (nb + 1) * 512)
                    nc.vector.tensor_copy(r_t[rrow:rrow + 1, ncol],
                                          pso[rrow:rrow + 1, :])
                for nb, pso in enumerate(halves):
                    ncol = slice(nb * 512, (nb + 1) * 512)
                    nc.vector.tensor_copy(vcp[off:off + 64, ncol],
                                          pso[off:off + 64, :])
                rd1 = dram_pool.tile([1, N], f32, tag="rd1")
                dq.dma_start(rd1, r_t[rrow:rrow + 1, :])
                rsq = r_pool.tile([P, N // P], f32, tag="rsq")
                dq.dma_start(
                    rsq, rd1.rearrange("one (p o) -> (one p) o", p=P))
                return rsq, vcp

            def norm_b(h, st, dq):
                """1/r on the repartitioned row; broadcast back across the
                64 output partitions (via DRAM round trip on dq)."""
                rsq, vcp = st
                off = (h % 2) * 64
                nc.vector.reciprocal(rsq, rsq)
                rd2 = dram_pool.tile([1, N], f32, tag="rd2")
                dq.dma_start(
                    rd2.rearrange("one (p o) -> (one p) o", p=P), rsq)
                rb = rb_pool.tile([P, N], f32, tag="rb")
                dq.dma_start(rb[off:off + 64, :],
                             rd2[0:1, :].partition_broadcast(64))
                return rb, vcp

            def norm_c(h, st):
                rb, vcp = st
                off = (h % 2) * 64
                for nb in range(NB2):
                    ncol = slice(nb * 512, (nb + 1) * 512)
                    nc.vector.tensor_mul(
                        oT_sbs[h // 2][off:off + 64, ncol],
                        vcp[off:off + 64, ncol],
                        rb[off:off + 64, ncol])

            # ---- the three pair windows -----------------------------------
            # pair 0: v-projection chunk per step + qkv m1 fills
            hooks0 = {sc: [lambda sc=sc: emit_v(sc)] for sc in range(SC)}
            hooks0[1].append(lambda: qkv_m1(1, "q"))
            hooks0[3].append(lambda: qkv_m1(1, "k"))
            ets0_e, ets0_o, _ = st_pair(0, hooks=hooks0)

            # pair 1: PV(pair 0) burst in steps 0-3; m2 fills at 4/6;
            # pair-0 norm chains staged over steps 4-7 (emitted inside the
            # window so they overlap it in the strict-FIFO engine queues).
            pv0, (h0_e, h0_o) = make_pv(0, [(0, ets0_e), (1, ets0_o)])
            st = {}
            hooks1 = {
                4: [lambda: st.__setitem__(0, norm_a(0, h0_e, nc.sync)),
                    lambda: qkv_m1(2, "q")],
                5: [lambda: st.__setitem__(0, norm_b(0, st[0], nc.sync)),
                    lambda: st.__setitem__(1, norm_a(1, h0_o, nc.sync))],
                6: [lambda: norm_c(0, st[0]),
                    lambda: st.__setitem__(1, norm_b(1, st[1], nc.sync)),
                    lambda: qkv_m1(2, "k")],
                7: [lambda: norm_c(1, st[1])],
            }
            ets1_e, ets1_o, _ = st_pair(
                2, pv_sched=[2, 2, 2, 2, 0, 0, 0, 0], pv_emit=pv0,
                hooks=hooks1)

            # pair 2: PV(pair 1) burst steps 0-3, pair-1 norms staged over
            # steps 4-7, pair 2's own PV (self-lag) from step 5 once the
            # ps_o slots have been evacuated by norm_a.
            pv1, (h1_e, h1_o) = make_pv(1, [(2, ets1_e), (3, ets1_o)])
            pv2_emit_holder = []

            def start_pv2():
                emit2, halves = make_pv(2, [(4, ets2_e), (5, ets2_o)])
                pv2_emit_holder.append((emit2, halves))

            ets2_e, ets2_o = [], []

            def pv2(i):
                pv2_emit_holder[0][0](i)

            hooks2 = {
                4: [lambda: st.__setitem__(2, norm_a(2, h1_e, nc.sync))],
                5: [lambda: st.__setitem__(2, norm_b(2, st[2], nc.sync)),
                    lambda: st.__setitem__(3, norm_a(3, h1_o, nc.sync)),
                    start_pv2, lambda: pv2(0)],
                6: [lambda: norm_c(2, st[2]),
                    lambda: st.__setitem__(3, norm_b(3, st[3], nc.sync)),
                    lambda: pv2(1), lambda: pv2(2)],
                7: [lambda: norm_c(3, st[3]),
                    lambda: pv2(3), lambda: pv2(4)],
            }
            st_pair(4, pv_sched=[2, 2, 2, 2, 0, 0, 0, 0], pv_emit=pv1,
                    hooks=hooks2, ets_out=(ets2_e, ets2_o))
            _, (h2_e, h2_o) = pv2_emit_holder[0]

            # ---- tail -----------------------------------------------------
            pv2_emit = pv2_emit_holder[0][0]
            for i in range(5, SC):
                pv2_emit(i)

            # oproj j3=0,1 pre-run for nb 0-3 (ps_s slots for 0-1, ps_o for
            # 2-3) while the final norm chains fly on sync+gpsimd.
            def oproj_mms(nb, ps0, ps1, j3s, start, stop):
                for cb, ps in ((0, ps0), (1, ps1)):
                    cw = 512 if cb == 0 else C - 512
                    for j3 in j3s:
                        mm(ps[:, 0:cw],
                           oT_sbs[j3][:, nb * P:(nb + 1) * P],
                           woT_sb[:, j3, cb * 512:cb * 512 + cw],
                           start=(start and j3 == j3s[0]),
                           stop=(stop and j3 == j3s[-1]))

            pre = {}
            for nb in range(2):
                psw = psum_s.tile([P, N], f32, tag="ps_s", name=f"pow_{nb}")
                pre[nb] = (psw[:, 0:512], psw[:, 512:1024])
                oproj_mms(nb, pre[nb][0], pre[nb][1], [0, 1], True, False)

            # final pair's norm: both chains in parallel (sync + gpsimd,
            # the ew stream is done), evacuations batched first so the
            # DVE FIFO never parks on an unresolved reciprocal before the
            # psum evacuations run.
            s4 = norm_a(4, h2_e, nc.sync)
            s5 = norm_a(5, h2_o, nc.gpsimd)

            for nb in range(2, 4):
                ps0 = psum_o.tile([P, 512], f32, tag="ps_o",
                                  name=f"po0_{nb}")
                ps1 = psum_o.tile([P, 512], f32, tag="ps_o",
                                  name=f"po1_{nb}")
                pre[nb] = (ps0, ps1)
                oproj_mms(nb, ps0, ps1, [0, 1], True, False)

            s4 = norm_b(4, s4, nc.sync)
            s5 = norm_b(5, s5, nc.gpsimd)
            norm_c(4, s4)
            norm_c(5, s5)

            def oproj_evac(nb, ps0, ps1):
                ob = out_pool.tile([P, C], f16, tag="ob")
                nc.scalar.copy(ob[:, 0:512], ps0)
                nc.scalar.copy(ob[:, 512:C], ps1[:, 0:C - 512])
                nc.sync.dma_start(
                    out_d.rearrange("(o p) c -> o p c", p=P)[nb], ob)

            for nb in range(SC):
                if nb in pre:
                    ps0, ps1 = pre[nb]
                    oproj_mms(nb, ps0, ps1, [2], False, True)
                else:
                    ps0 = psum_o.tile([P, 512], f32, tag="ps_o",
                                      name=f"po0_{nb}")
                    ps1 = psum_o.tile([P, 512], f32, tag="ps_o",
                                      name=f"po1_{nb}")
                    oproj_mms(nb, ps0, ps1, [0, 1, 2], True, True)
                oproj_evac(nb, ps0, ps1)

    nc.compile()
    return nc


_PROG = None


def _get_prog():
    global _PROG
    if _PROG is None:
        _PROG = build_program()
    return _PROG


def make_in_maps(query, attn_weight, Wq, Wk, Wv, Wo):
    query = np.asarray(query, dtype=np.float32)
    attn_weight = np.asarray(attn_weight, dtype=np.float32)
    Wq = np.asarray(Wq, dtype=np.float32)
    Wk = np.asarray(Wk, dtype=np.float32)
    Wv = np.asarray(Wv, dtype=np.float32)
    Wo = np.asarray(Wo, dtype=np.float32)

    in_maps = []
    for b in range(B):
        xT = np.ascontiguousarray(query[b].T).astype(np.float16)
        for g in range(HG):
            rows = slice(g * GJ, (g + 1) * GJ)
            wqk = np.ascontiguousarray(np.concatenate(
                [(SCALE * Wq[rows, :]).T, Wk[rows, :].T],
                axis=1)).astype(np.float16)
            wvT = np.ascontiguousarray(Wv[rows, :].T).astype(np.float16)
            woT = np.ascontiguousarray(Wo[:, rows].T).astype(np.float16)
            ew = np.exp(np.ascontiguousarray(
                attn_weight[b, g * HPG:(g + 1) * HPG].transpose(0, 2, 1))
            ).astype(np.float16)
            in_maps.append({
                "xT": xT, "wqk": wqk, "wvT": wvT, "woT": woT, "ew": ew,
            })
    return in_maps


def run(inputs, trace=False, **spmd_kwargs):
    """Execute on 8 cores; returns (full_output, BassKernelResults)."""
    from concourse import bass_utils

    nc = _get_prog()
    in_maps = make_in_maps(inputs["query"], inputs["attn_weight"],
                           inputs["Wq"], inputs["Wk"], inputs["Wv"],
                           inputs["Wo"])
    res = bass_utils.run_bass_kernel_spmd(
        nc, in_maps, core_ids=list(range(NCORES)), trace=trace, **spmd_kwargs)
    bo = np.asarray(inputs["bo"], dtype=np.float32)
    full = np.empty((B, N, C), dtype=np.float32)
    for b in range(B):
        full[b] = (res.results[2 * b]["out"].astype(np.float32)
                   + res.results[2 * b + 1]["out"].astype(np.float32) + bo)
    return full, res


def kernel(**inputs):
    full, _ = run(inputs, trace=False)
    return full


# revision 15
# speedup vs baseline: 1.1476x; 1.1137x over previous
"""Trainium2 Bass kernel for a dense self-attention block (B=4, N=S=1024,
C=768, H=12) with an additive attention-weight bias:

    q = heads(x @ Wq.T); k = heads(x @ Wk.T); v = heads(x @ Wv.T)
    attn = softmax(attn_weight + log_softmax(scale * q k^T))
    out  = (attn @ v) @ Wo.T + bo

Math simplifications (exact):
  softmax(w + log_softmax(a)) == softmax(w + a)          (lse shift invariance)
  exp(w + a) == exp(a) * exp(w)  with exp(w) precomputed on HOST.

The second identity removes the in-PE bias-add (an identity-matmul per
attention tile that cost ~25% of all TensorE columns in v1): the device
computes et = exp(qk) on ACT and multiplies elementwise by the streamed
exp(w) tiles on the otherwise-idle Vector engine.

Scheduling principle (HAM): the PE clock-gate only stays at 2.4 GHz while
the PE is ~fully busy, so every S^T window is packed with filler matmul
work (v-projection, qkv m1/m2 chunks, PV bursts) to keep the TensorE FIFO
nonempty; emission order per step is [dense fills] -> qk -> PV -> exp ->
mul -> [late-dependency fills] so a fill waiting on a startup DMA can
never head-of-line-block the qk stream.

Norms: 1/r for pairs 0/1 via the DMA repartition chain (≈11us latency,
hidden: launched 1-2 windows before the result is needed); the final pair
uses the DMA-free exp(-ln r) ACT path + K=1 ones-matmul broadcast.

Sharding: 8 cores = 4 batches x 2 head-groups (6 heads each); host sums
the two half-projections + bias in fp32.
"""

import os
import numpy as np

B, N, C, H = 4, 1024, 768, 12
HG = 2                # head-groups (tensor-parallel factor); cores = B*HG = 8
HPG = H // HG         # heads per group = 6
D = C // H            # 64
GJ = HPG * D          # 384
P = 128
SC = N // P           # 8 s-chunks of 128
MQ = GJ // P          # 3 row chunks of qT/kT
KC = C // P           # 6 contraction chunks over C
NB2 = N // 512        # 2 column chunks of 512
NCORES = B * HG
SCALE = D ** -0.5

NWARM = int(os.environ.get("K_NWARM", "44"))
EW_BUFS = 9
ET_BUFS = 18
ERAW_BUFS = 6


def build_program():
    """Build and compile the per-core Bass program. Returns the Bacc object."""
    import concourse.bass as bass
    import concourse.mybir as mybir
    import concourse.tile as tile
    from concourse import bacc

    nc = bacc.Bacc(
        "TRN2",
        target_bir_lowering=False,
        debug=False,
        num_devices=NCORES,
    )
    f32 = mybir.dt.float32
    f16 = mybir.dt.float16
    EXP = mybir.ActivationFunctionType.Exp
    LOG = mybir.ActivationFunctionType.Ln

    xT_d = nc.dram_tensor("xT", [C, N], f16, kind="ExternalInput").ap()
    wqk_d = nc.dram_tensor("wqk", [C, 2 * GJ], f16, kind="ExternalInput").ap()
    wvT_d = nc.dram_tensor("wvT", [C, GJ], f16, kind="ExternalInput").ap()
    woT_d = nc.dram_tensor("woT", [GJ, C], f16, kind="ExternalInput").ap()
    ew_d = nc.dram_tensor("ew", [HPG, N, N], f16, kind="ExternalInput").ap()
    out_d = nc.dram_tensor("out", [N, C], f16, kind="ExternalOutput").ap()

    def mm(out, lhsT, rhs, start, stop):
        nc.tensor.matmul(out, lhsT, rhs, start=start, stop=stop)

    with tile.TileContext(nc) as tc:
        with (
            tc.tile_pool(name="const", bufs=1) as const_pool,
            tc.tile_pool(name="ewtile", bufs=EW_BUFS) as ew_pool,
            tc.tile_pool(name="eraw", bufs=ERAW_BUFS) as eraw_pool,
            tc.tile_pool(name="etile", bufs=ET_BUFS) as e_pool,
            tc.tile_pool(name="rtile", bufs=4) as r_pool,
            tc.tile_pool(name="rbtile", bufs=2) as rb_pool,
            tc.tile_pool(name="vcptile", bufs=3) as vcp_pool,
            tc.tile_pool(name="outtile", bufs=2) as out_pool,
            tc.tile_pool(name="ps_s", bufs=2, space="PSUM") as psum_s,
            tc.tile_pool(name="ps_o", bufs=4, space="PSUM") as psum_o,
            tc.tile_pool(name="dram", bufs=4, space="DRAM") as dram_pool,
        ):
            # ---- constants / fill -----------------------------------------
            # x / wqk in 3-chunk groups (768/576 KB transfers: big enough
            # for ~75% DMA efficiency), interleaved g0-first on two queues
            # so the first contraction chunks land earliest and the QKV-m0
            # matmuls stream right behind the fill.
            warm_sb = const_pool.tile([P, P], f16)
            nc.gpsimd.memset(warm_sb, 0.0)
            ones_sb = const_pool.tile([1, P], f16)
            nc.gpsimd.memset(ones_sb, 1.0)

            xg = [const_pool.tile([P, 3, N], f16, name=f"xg{g}")
                  for g in range(2)]
            wqkg = [const_pool.tile([P, 3, 2 * GJ], f16, name=f"wqkg{g}")
                    for g in range(2)]
            xT_r = xT_d.rearrange("(o p) n -> p o n", p=P)
            wqk_r = wqk_d.rearrange("(o p) j -> p o j", p=P)
            for g in range(2):
                nc.sync.dma_start(xg[g], xT_r[:, 3 * g:3 * g + 3])
                nc.scalar.dma_start(wqkg[g], wqk_r[:, 3 * g:3 * g + 3])
            x_sbs = [xg[k // 3][:, k % 3, :] for k in range(KC)]
            wqk_sbs = [wqkg[k // 3][:, k % 3, :] for k in range(KC)]

            # wv / woT on the sync queue, gated behind the last x group (a
            # tiny DMA with a real data dependency — the list scheduler
            # cannot hoist the loads ahead of the fill, and same-queue
            # FIFO then orders the transfers after the gate fires).
            wvg = [const_pool.tile([P, 3, GJ], f16, name=f"wvg{g}")
                   for g in range(2)]
            woT_sb = const_pool.tile([P, MQ, C], f16)
            nc.sync.dma_start(wvg[0][0:1, 0:1, 0:64], xg[1][0:1, 0, 0:64])
            wv_r = wvT_d.rearrange("(o p) j -> p o j", p=P)
            for g in range(2):
                nc.sync.dma_start(wvg[g], wv_r[:, 3 * g:3 * g + 3])
            nc.sync.dma_start(woT_sb,
                              woT_d.rearrange("(o p) c -> p o c", p=P))
            wv_sbs = [wvg[k // 3][:, k % 3, :] for k in range(KC)]

            # gate the ew stream the same way (dummy pool tiles; real ew
            # DMAs rotate into these slots, semaphore-ordered after fill)
            for i in range(EW_BUFS):
                g = ew_pool.tile([P, 2, N], f16, tag="ew", name=f"ewgate{i}")
                nc.gpsimd.dma_start(g[0:1, 0:1, 0:64], xg[1][0:1, 0, 0:64])
            # ew view: [sc, p, h, n]
            ew_r = ew_d.rearrange("h (c p) n -> c p h n", p=P)

            # ---- PE warmup ------------------------------------------------
            # Zero-matmul stream covering preamble -> first-group landing,
            # so the HAM clock-gate un-throttles into the QKV-m0 stream.
            warm_ps = psum_s.tile([P, N], f32, tag="ps_s")
            for i in range(NWARM):
                mm(warm_ps[:, 0:P], warm_sb, warm_sb,
                   start=(i == 0), stop=(i == NWARM - 1))

            qT_sbs = [const_pool.tile([P, N], f16, name=f"qT{j}")
                      for j in range(MQ)]
            kT_sbs = [const_pool.tile([P, N], f16, name=f"kT{j}")
                      for j in range(MQ)]
            oT_sbs = [const_pool.tile([P, N], f16, name=f"oT{j}")
                      for j in range(MQ)]
            # [v_h | 1 | 0...] (even heads use cols 0:65) /
            # [0... | 1 | 0 | v_h] (odd heads use cols 0:128, one at col 32)
            v_aug = const_pool.tile([P, SC, HPG, P], f16)
            for h in range(HPG):
                if h % 2 == 0:
                    nc.scalar.memzero(v_aug[:, :, h, 64:66])
                    nc.scalar.add(v_aug[:, :, h, 64:65],
                                  v_aug[:, :, h, 64:65], 1.0)
                else:
                    nc.scalar.memzero(v_aug[:, :, h, 0:64])
                    nc.scalar.add(v_aug[:, :, h, 32:33],
                                  v_aug[:, :, h, 32:33], 1.0)

            # ---- QKV projections ------------------------------------------
            # m0 streams behind the fill; casts split in halves so pair 0's
            # first steps unblock one ACT-copy earlier.
            def qkv_m0():
                ps_q = psum_s.tile([P, N], f32, tag="ps_s", name="qkv_q0")
                ps_k = psum_s.tile([P, N], f32, tag="ps_s", name="qkv_k0")
                for kc in range(KC):
                    for ps, j0 in ((ps_q, 0), (ps_k, GJ)):
                        for nb in range(NB2):
                            ncol = slice(nb * 512, (nb + 1) * 512)
                            mm(ps[:, ncol],
                               wqk_sbs[kc][:, j0:j0 + P],
                               x_sbs[kc][:, ncol],
                               start=(kc == 0), stop=(kc == KC - 1))
                # NB: DVE CAST (f32 psum -> f16) mis-strides on HW;
                # ScalarE casts fine.
                for half in range(NB2):
                    ncol = slice(half * 512, (half + 1) * 512)
                    nc.scalar.copy(kT_sbs[0][:, ncol], ps_k[:, ncol])
                    nc.scalar.copy(qT_sbs[0][:, ncol], ps_q[:, ncol])

            qkv_m0()

            def qkv_m1(m, which):
                """Emit one of q/k for row-chunk m (1 psum slot borrow)."""
                j0 = m * P if which == "q" else GJ + m * P
                dst = qT_sbs[m] if which == "q" else kT_sbs[m]
                ps = psum_s.tile([P, N], f32, tag="ps_s",
                                 name=f"qkv_{which}{m}")
                for nb in range(NB2):
                    ncol = slice(nb * 512, (nb + 1) * 512)
                    for kc in range(KC):
                        mm(ps[:, ncol], wqk_sbs[kc][:, j0:j0 + P],
                           x_sbs[kc][:, ncol],
                           start=(kc == 0), stop=(kc == KC - 1))
                nc.scalar.copy(dst[:], ps)

            def emit_v(sc):
                # v-projection for one s-chunk on a ps_o slot; ScalarE
                # scatters the result into v_aug (even cols 0:64 / odd
                # cols 64:128 per head).
                ps = psum_o.tile([P, 512], f32, tag="ps_o", name=f"ps_v{sc}")
                for kc in range(KC):
                    mm(ps[:, :GJ],
                       x_sbs[kc][:, sc * P:(sc + 1) * P],
                       wv_sbs[kc][:, :],
                       start=(kc == 0), stop=(kc == KC - 1))
                vsrc = ps[:, :GJ].rearrange("p (h d) -> p h d", d=D)
                nc.scalar.copy(v_aug[:, sc, 0:HPG:2, 0:64],
                               vsrc[:, 0:HPG:2, :])
                nc.scalar.copy(v_aug[:, sc, 1:HPG:2, 64:128],
                               vsrc[:, 1:HPG:2, :])

            # ---- attention pair loop --------------------------------------
            def st_pair(hp, pv_sched=None, pv_emit=None, hooks_pre=None,
                        hooks=None, ets_out=None, prefetch_ew=False):
                """One even/odd head pair's qk -> exp -> (x ew) stream.
                hooks_pre[sc]: dense PE fills emitted BEFORE the qk mms
                (must not depend on late startup DMAs).  hooks[sc]: work
                emitted after the step's mul (may have late deps).
                pv_sched[sc] PV groups of pv_emit interleave after qk."""
                j = hp // 2
                qe, ke = qT_sbs[j][0:64, :], kT_sbs[j][0:64, :]
                qo, ko = qT_sbs[j][64:128, :], kT_sbs[j][64:128, :]
                ets_e, ets_o = ([], []) if ets_out is None else ets_out
                pv_next = 0
                ew_ts = []
                if prefetch_ew:
                    for sc in range(SC):
                        ew_t = ew_pool.tile([P, 2, N], f16, tag="ew")
                        nc.gpsimd.dma_start(ew_t, ew_r[sc][:, hp:hp + 2, :])
                        ew_ts.append(ew_t)
                for sc in range(SC):
                    scol = slice(sc * P, (sc + 1) * P)
                    if prefetch_ew:
                        ew_t = ew_ts[sc]
                    else:
                        ew_t = ew_pool.tile([P, 2, N], f16, tag="ew")
                        nc.gpsimd.dma_start(ew_t, ew_r[sc][:, hp:hp + 2, :])
                    if hooks_pre is not None and sc in hooks_pre:
                        for fn in hooks_pre[sc]:
                            fn()
                    ps_e = psum_s.tile([P, N], f32, tag="ps_s")
                    ps_o = psum_s.tile([P, N], f32, tag="ps_s")
                    # each 512-col psum region is written by exactly one
                    # matmul -> start+stop per region
                    for nb in range(NB2):
                        ncol = slice(nb * 512, (nb + 1) * 512)
                        mm(ps_e[:, ncol], ke[:, scol], qe[:, ncol],
                           start=True, stop=True)
                    for nb in range(NB2):
                        ncol = slice(nb * 512, (nb + 1) * 512)
                        mm(ps_o[:, ncol], ko[:, scol], qo[:, ncol],
                           start=True, stop=True)
                    if pv_emit is not None and pv_sched is not None:
                        for _ in range(pv_sched[sc]):
                            pv_emit(pv_next)
                            pv_next += 1
                    er_e = eraw_pool.tile([P, N], f16, tag="eraw")
                    nc.scalar.activation(er_e, ps_e, EXP)
                    er_o = eraw_pool.tile([P, N], f16, tag="eraw")
                    nc.scalar.activation(er_o, ps_o, EXP)
                    et_e = e_pool.tile([P, N], f16, tag="et")
                    nc.vector.tensor_mul(et_e, er_e, ew_t[:, 0, :])
                    et_o = e_pool.tile([P, N], f16, tag="et")
                    nc.vector.tensor_mul(et_o, er_o, ew_t[:, 1, :])
                    ets_e.append(et_e)
                    ets_o.append(et_o)
                    if hooks is not None and sc in hooks:
                        for fn in hooks[sc]:
                            fn()
                return ets_e, ets_o, pv_next

            def make_pv(hp, ets_pair):
                halves = {}
                for h, _ in ets_pair:
                    for nb in range(NB2):
                        halves[(h, nb)] = psum_o.tile([P, 512], f32,
                                                      tag="ps_o",
                                                      name=f"pso_h{h}_n{nb}")

                def emit(sc):
                    for h, etiles in ets_pair:
                        even = (h % 2 == 0)
                        lh = (v_aug[:, sc, h, 0:65] if even
                              else v_aug[:, sc, h, 0:P])
                        for nb in range(NB2):
                            ncol = slice(nb * 512, (nb + 1) * 512)
                            pso = halves[(h, nb)]
                            po = (pso[0:65, :] if even else pso[:, :])
                            mm(po, lh, etiles[sc][:, ncol],
                               start=(sc == 0), stop=(sc == SC - 1))

                h_e, h_o = ets_pair[0][0], ets_pair[1][0]
                return emit, ([halves[(h_e, 0)], halves[(h_e, 1)]],
                              [halves[(h_o, 0)], halves[(h_o, 1)]])

            # Norm chain (pairs 0/1), split into 3 stages so each DVE op's
            # inputs are resolved before it reaches the head of the
            # strict-FIFO DVE queue (an unresolved reciprocal would block
            # the exp-tile multiply stream for ~2us).  End-to-end latency
            # is ~11us (4 HBM round trips) — launched 1.5 windows before
            # the tail needs oT.
            def norm_a(h, halves, dq):
                off = (h % 2) * 64
                rrow = 64 if h % 2 == 0 else 32
                r_t = r_pool.tile([P, N], f32, tag="r")
                vcp = vcp_pool.tile([P, N], f32, tag="vcp")
                for nb, pso in enumerate(halves):
                    ncol = slice(nb * 512, (nb + 1) * 512)
                    nc.vector.tensor_copy(r_t[rrow:rrow + 1, ncol],
                                          pso[rrow:rrow + 1, :])
                for nb, pso in enumerate(halves):
                    ncol = slice(nb * 512, (nb + 1) * 512)
                    nc.vector.tensor_copy(vcp[off:off + 64, ncol],
                                          pso[off:off + 64, :])
                rd1 = dram_pool.tile([1, N], f32, tag="rd1")
                dq.dma_start(rd1, r_t[rrow:rrow + 1, :])
                rsq = r_pool.tile([P, N // P], f32, tag="rsq")
                dq.dma_start(
                    rsq, rd1.rearrange("one (p o) -> (one p) o", p=P))
                return rsq, vcp

            def norm_b(h, st, dq):
                rsq, vcp = st
                off = (h % 2) * 64
                nc.vector.reciprocal(rsq, rsq)
                rd2 = dram_pool.tile([1, N], f32, tag="rd2")
                dq.dma_start(
                    rd2.rearrange("one (p o) -> (one p) o", p=P), rsq)
                rb = rb_pool.tile([P, N], f32, tag="rb")
                dq.dma_start(rb[off:off + 64, :],
                             rd2[0:1, :].partition_broadcast(64))
                return rb, vcp

            def norm_c(h, st):
                rb, vcp = st
                off = (h % 2) * 64
                for nb in range(NB2):
                    ncol = slice(nb * 512, (nb + 1) * 512)
                    nc.vector.tensor_mul(
                        oT_sbs[h // 2][off:off + 64, ncol],
                        vcp[off:off + 64, ncol],
                        rb[off:off + 64, ncol])

            # ---- the three pair windows -----------------------------------
            # pair 0 (heads 0/1): m1 fills dense-early, v-projection per
            # step as a post-hook (wv lands late in the startup fill; the
            # qk stream must not queue behind it).
            hooks0_pre = {2: [lambda: qkv_m1(1, "q")],
                          5: [lambda: qkv_m1(1, "k")]}
            hooks0 = {sc: [lambda sc=sc: emit_v(sc)] for sc in range(SC)}
            ets0_e, ets0_o, _ = st_pair(0, hooks_pre=hooks0_pre,
                                        hooks=hooks0)

            # pair 1 (heads 2/3): PV(pair 0) spread (closes step 5), m2
            # fills at 5/6, pair-0 norm chains start at 6/7 (their ~11us
            # DMA latency resolves mid-pair-2, well before the tail).
            st = {}
            pv0, (h0_e, h0_o) = make_pv(0, [(0, ets0_e), (1, ets0_o)])
            hooks1_pre = {5: [lambda: qkv_m1(2, "q")],
                          6: [lambda: qkv_m1(2, "k")]}
            hooks1 = {
                6: [lambda: st.__setitem__(0, norm_a(0, h0_e, nc.sync))],
                7: [lambda: st.__setitem__(0, norm_b(0, st[0], nc.sync)),
                    lambda: st.__setitem__(1, norm_a(1, h0_o, nc.sync))],
            }
            ets1_e, ets1_o, _ = st_pair(
                2, pv_sched=[2, 2, 1, 1, 1, 1, 0, 0], pv_emit=pv0,
                hooks_pre=hooks1_pre, hooks=hooks1)

            # pair 2 (heads 4/5): all 8 ew tiles prefetched at step 0 (the
            # gpsimd queue is then free for pair-1's even norm chain —
            # chain hops never head-of-line-block the ew stream).  PV(1)
            # burst closes at step 3; pair-1 norms staged 4-7 (sync +
            # gpsimd in parallel, landing ~tail+1); pair-2's own PV
            # self-lags from step 5 into the freed ps_o slots.
            pv1, (h1_e, h1_o) = make_pv(1, [(2, ets1_e), (3, ets1_o)])
            pv2_emit_holder = []

            def start_pv2():
                emit2, halves = make_pv(2, [(4, ets2_e), (5, ets2_o)])
                pv2_emit_holder.append((emit2, halves))

            ets2_e, ets2_o = [], []

            def pv2(i):
                pv2_emit_holder[0][0](i)

            hooks2 = {
                0: [lambda: norm_c(0, st[0]),
                    lambda: st.__setitem__(1, norm_b(1, st[1], nc.sync))],
                1: [lambda: norm_c(1, st[1])],
                4: [lambda: st.__setitem__(2, norm_a(2, h1_e, nc.gpsimd))],
                5: [lambda: st.__setitem__(2, norm_b(2, st[2], nc.gpsimd)),
                    lambda: st.__setitem__(3, norm_a(3, h1_o, nc.sync)),
                    start_pv2, lambda: pv2(0)],
                6: [lambda: norm_c(2, st[2]),
                    lambda: st.__setitem__(3, norm_b(3, st[3], nc.sync)),
                    lambda: pv2(1), lambda: pv2(2)],
                7: [lambda: norm_c(3, st[3]),
                    lambda: pv2(3), lambda: pv2(4)],
            }
            st_pair(4, pv_sched=[2, 2, 2, 2, 0, 0, 0, 0], pv_emit=pv1,
                    hooks=hooks2, ets_out=(ets2_e, ets2_o),
                    prefetch_ew=True)
            _, (h2_e, h2_o) = pv2_emit_holder[0]

            # ---- tail -----------------------------------------------------
            pv2_emit = pv2_emit_holder[0][0]
            for i in range(5, SC):
                pv2_emit(i)

            def oproj_mms(nb, ps0, ps1, j3s, start, stop):
                for cb, ps in ((0, ps0), (1, ps1)):
                    cw = 512 if cb == 0 else C - 512
                    for j3 in j3s:
                        mm(ps[:, 0:cw],
                           oT_sbs[j3][:, nb * P:(nb + 1) * P],
                           woT_sb[:, j3, cb * 512:cb * 512 + cw],
                           start=(start and j3 == j3s[0]),
                           stop=(stop and j3 == j3s[-1]))

            # oproj j3=0,1 pre-run for nb 0/1 on the freed ps_s slots: PE
            # work that covers the final norm's ACT latency.
            pre = {}
            for nb in range(2):
                psw = psum_s.tile([P, N], f32, tag="ps_s", name=f"pow_{nb}")
                pre[nb] = (psw[:, 0:512], psw[:, 512:1024])
                oproj_mms(nb, pre[nb][0], pre[nb][1], [0, 1], True, False)

            # Final pair's norm, DMA-free: 1/r = exp(-ln r) on ACT (Ln and
            # Exp cost one table-set switch each, ~1.3us — still ~5us
            # cheaper than the DMA chain's HBM round trips), then a K=1
            # ones-matmul broadcasts 1/r across the 64 output partitions.
            # All psum evacuations first: frees the ps_o slots for the
            # rbp broadcasts and the remaining oproj tiles.
            infos = []
            for h, halves in ((4, h2_e), (5, h2_o)):
                off = (h % 2) * 64
                rrow = 64 if h % 2 == 0 else 32
                r_t = r_pool.tile([P, N], f32, tag="r", name=f"rt{h}")
                for nb, pso in enumerate(halves):
                    ncol = slice(nb * 512, (nb + 1) * 512)
                    nc.vector.tensor_copy(r_t[rrow:rrow + 1, ncol],
                                          pso[rrow:rrow + 1, :])
                infos.append((h, off, rrow, r_t))
            vcps = {}
            for (h, off, rrow, r_t), halves in zip(infos, (h2_e, h2_o)):
                vcp = vcp_pool.tile([P, N], f32, tag="vcp", name=f"vcpt{h}")
                for nb, pso in enumerate(halves):
                    ncol = slice(nb * 512, (nb + 1) * 512)
                    nc.vector.tensor_copy(vcp[off:off + 64, ncol],
                                          pso[off:off + 64, :])
                vcps[h] = vcp
            lns = []
            for h, off, rrow, r_t in infos:
                rln = r_pool.tile([1, N], f32, tag="rsq", name=f"rln{h}")
                nc.scalar.activation(rln, r_t[rrow:rrow + 1, :], LOG)
                lns.append(rln)
            rinvs = []
            for (h, off, rrow, r_t), rln in zip(infos, lns):
                rinv = r_pool.tile([1, N], f16, tag="rfl", name=f"rinv{h}")
                nc.scalar.activation(rinv, rln, EXP, scale=-1.0)
                rinvs.append(rinv)
            for (h, off, rrow, r_t), rinv in zip(infos, rinvs):
                for nb in range(NB2):
                    ncol = slice(nb * 512, (nb + 1) * 512)
                    rbp = psum_o.tile([P, 512], f32, tag="ps_o",
                                      name=f"rbp{h}_{nb}")
                    mm(rbp[off:off + 64, :], ones_sb[0:1, 0:64],
                       rinv[0:1, ncol], start=True, stop=True)
                    nc.vector.tensor_mul(
                        oT_sbs[h // 2][off:off + 64, ncol],
                        vcps[h][off:off + 64, ncol],
                        rbp[off:off + 64, :])

            def oproj_evac(nb, ps0, ps1):
                ob = out_pool.tile([P, C], f16, tag="ob")
                nc.scalar.copy(ob[:, 0:512], ps0)
                nc.scalar.copy(ob[:, 512:C], ps1[:, 0:C - 512])
                nc.sync.dma_start(
                    out_d.rearrange("(o p) c -> o p c", p=P)[nb], ob)

            for nb in range(SC):
                if nb in pre:
                    ps0, ps1 = pre[nb]
                    oproj_mms(nb, ps0, ps1, [2], False, True)
                else:
                    ps0 = psum_o.tile([P, 512], f32, tag="ps_o",
                                      name=f"po0_{nb}")
                    ps1 = psum_o.tile([P, 512], f32, tag="ps_o",
                                      name=f"po1_{nb}")
                    oproj_mms(nb, ps0, ps1, [0, 1, 2], True, True)
                oproj_evac(nb, ps0, ps1)

    nc.compile()
    return nc


_PROG = None


def _get_prog():
    global _PROG
    if _PROG is None:
        _PROG = build_program()
    return _PROG


def make_in_maps(query, attn_weight, Wq, Wk, Wv, Wo):
    query = np.asarray(query, dtype=np.float32)
    attn_weight = np.asarray(attn_weight, dtype=np.float32)
    Wq = np.asarray(Wq, dtype=np.float32)
    Wk = np.asarray(Wk, dtype=np.float32)
    Wv = np.asarray(Wv, dtype=np.float32)
    Wo = np.asarray(Wo, dtype=np.float32)

    in_maps = []
    for b in range(B):
        xT = np.ascontiguousarray(query[b].T).astype(np.float16)
        for g in range(HG):
            rows = slice(g * GJ, (g + 1) * GJ)
            wqk = np.ascontiguousarray(np.concatenate(
                [(SCALE * Wq[rows, :]).T, Wk[rows, :].T],
                axis=1)).astype(np.float16)
            wvT = np.ascontiguousarray(Wv[rows, :].T).astype(np.float16)
            woT = np.ascontiguousarray(Wo[:, rows].T).astype(np.float16)
            ew = np.exp(np.ascontiguousarray(
                attn_weight[b, g * HPG:(g + 1) * HPG].transpose(0, 2, 1))
            ).astype(np.float16)
            in_maps.append({
                "xT": xT, "wqk": wqk, "wvT": wvT, "woT": woT, "ew": ew,
            })
    return in_maps


def run(inputs, trace=False, **spmd_kwargs):
    """Execute on 8 cores; returns (full_output, BassKernelResults)."""
    from concourse import bass_utils

    nc = _get_prog()
    in_maps = make_in_maps(inputs["query"], inputs["attn_weight"],
                           inputs["Wq"], inputs["Wk"], inputs["Wv"],
                           inputs["Wo"])
    res = bass_utils.run_bass_kernel_spmd(
        nc, in_maps, core_ids=list(range(NCORES)), trace=trace, **spmd_kwargs)
    bo = np.asarray(inputs["bo"], dtype=np.float32)
    full = np.empty((B, N, C), dtype=np.float32)
    for b in range(B):
        full[b] = (res.results[2 * b]["out"].astype(np.float32)
                   + res.results[2 * b + 1]["out"].astype(np.float32) + bo)
    return full, res


def kernel(**inputs):
    full, _ = run(inputs, trace=False)
    return full
